# revision 1
# baseline (speedup 1.0000x reference)
"""HOG generator kernel for Trainium2, data-parallel over 8 NeuronCores.

Algorithm (per image, validated against the jax reference in numpy):
  - Sobel gx/gy as separable convs: horizontal part on DVE (shifted APs,
    reflect edge cols exact-zero / doubled), vertical part on PE via banded
    113x112 matrices that fold in reflect padding.
  - Orientation binning without atan2: bin boundaries k*pi/9 become sign
    tests of q_k = +-(A - tan_k*B), A = gx^2, B = gx*gy.  Cumulative masked
    magnitudes t_k = magG * [q_k > 0] via ACT Sigmoid(q*1e30 - 40) (exact
    0/1 off the boundary sliver) times magG on DVE.
  - Gaussian weighting folded into the magnitude: row factor via ACT Sqrt
    scale, column factor via one TT multiply with a constant tile.
  - 8x8 pooling: columns via strided tensor_reduce, rows via PE pool matmul.
  - Bin histograms = adjacent differences of the cumulative pools, then
    L2-normalized over the 9 bins.  Device output is (img, 28, 9, 28);
    the final (b, 196, 36) unfold permutation happens on the host.
"""
import math
import sys

import numpy as np

sys.path.insert(0, "/opt/trn_rl_repo")

import concourse.bass as bass
import concourse.bacc as bacc
import concourse.mybir as mybir
from concourse import tile
from concourse.bass_utils import run_bass_kernel_spmd

N_CORES = 8
IMGS_PER_CORE = 16
H = W = 224
NB = 9
F32 = mybir.dt.float32
BF16 = mybir.dt.bfloat16
AF = mybir.ActivationFunctionType
OP = mybir.AluOpType
TANS = [math.tan(k * math.pi / 9.0) for k in range(1, 9)]


def _host_constants(weight_x, gaussian_kernel):
    """Derive the device constant tensors from the module inputs."""
    wx = np.asarray(weight_x, np.float32).reshape(3, 3)
    v_s = wx[:, 0].copy()                      # [1,2,1] vertical smooth
    v_d = wx[0, :].copy()                      # [1,0,-1] -> vertical diff vec
    g2 = np.asarray(gaussian_kernel, np.float64).reshape(16, 16)
    wt = np.sqrt(np.diag(g2)).astype(np.float32)   # g2[i,j] == wt[i]*wt[j]

    def band(chunk, vec):
        m = np.zeros((113, 112), np.float32)
        for i in range(112):
            for d in range(3):
                if chunk == 0:
                    r = i - 1 + d
                    if r == -1:
                        r = 1
                else:
                    r = i + d
                    if r == 113:
                        r = 111
                m[r, i] += vec[d]
        return m

    poolm = np.zeros((112, 14), np.float32)
    for r in range(112):
        poolm[r, r // 8] = 1.0

    blob = np.zeros((113, 689), np.float32)
    blob[:, 0:112] = band(0, v_s)
    blob[:, 112:224] = band(1, v_s)
    blob[:, 224:336] = band(0, v_d)
    blob[:, 336:448] = band(1, v_d)
    blob[0:112, 448:462] = poolm
    blob[:, 462:686] = wt[np.arange(224) % 16][None, :]
    blob[0:112, 686] = wt[np.arange(112) % 16] ** 2
    blob[:, 687] = 0.0
    blob[:, 688] = -40.0
    return {"consts": blob}


def _rep(ap, n, pos=1):
    """Insert a broadcast (step-0) dim of size n into an AP at free pos."""
    import copy
    l = [list(d) for d in ap.ap]
    l.insert(pos, [0, n])
    return bass.AP(ap.tensor, ap.offset, l)


def build_program(n_img=IMGS_PER_CORE):
    assert n_img % 2 == 0
    nc = bacc.Bacc("TRN2", debug=False)
    x_d = nc.dram_tensor("x", [n_img, 224, 224], F32, kind="ExternalInput").ap()
    const_d = nc.dram_tensor("consts", [113, 689], F32, kind="ExternalInput").ap()
    out_d = nc.dram_tensor("out", [n_img, 28, NB, 28], F32, kind="ExternalOutput").ap()
    AX = mybir.AxisListType.X

    with tile.TileContext(nc) as tc:
        with (
            tc.tile_pool(name="const", bufs=1) as cp,
            tc.tile_pool(name="work", bufs=3) as wp,
            tc.tile_pool(name="small", bufs=3) as sp,
            tc.tile_pool(name="psum", bufs=2, space="PSUM") as pp,
            tc.tile_pool(name="psum2", bufs=2, space="PSUM") as pp2,
        ):
            CT = cp.tile([113, 689], F32, tag="CT")
            nc.sync.dma_start(CT[:, :], const_d)
            lhs_s = [CT[:, 0:112], CT[:, 112:224]]
            lhs_d = [CT[:, 224:336], CT[:, 336:448]]
            poolm_ap = CT[0:112, 448:462]
            gc_ap = CT[0:112, 462:686]
            gr2_ap = CT[0:112, 686:687]
            zb = CT[:, 687:688]
            nb40 = CT[:, 688:689]

            pending = [None]

            def flush_norm():
                if pending[0] is None:
                    return
                Hh, ss, i0, ch = pending[0]
                pending[0] = None
                nrm = sp.tile([14, 56], F32, tag="nrm")
                nc.scalar.activation(nrm[:, :], ss[:, :], AF.Sqrt,
                                     bias=zb[0:14, 0:1])
                nc.vector.tensor_scalar_max(nrm[:, :], nrm[:, :], 1e-12)
                inv = sp.tile([14, 56], F32, tag="inv")
                nc.vector.reciprocal(inv[:, :], nrm[:, :])
                OUT = sp.tile([14, NB * 56], F32, tag="OUT")
                hv = Hh[:, :].rearrange("p (i k c) -> p i k c", i=2, k=NB)
                ov = OUT[:, :].rearrange("p (i k c) -> p i k c", i=2, k=NB)
                iv = _rep(inv[:, :].rearrange("p (i c) -> p i c", i=2), NB, pos=2)
                nc.vector.tensor_mul(ov, hv, iv)
                nc.gpsimd.dma_start(
                    out_d[i0:i0 + 2, ch * 14:(ch + 1) * 14, :, :]
                    .rearrange("i r k c -> r i k c"), ov)

            for i0 in range(0, n_img, 2):
                for ch in range(2):
                    r0 = 0 if ch == 0 else 111
                    X = wp.tile([113, 448], F32, tag="X")
                    nc.sync.dma_start(X[:, 0:224], x_d[i0, r0:r0 + 113, :])
                    nc.scalar.dma_start(X[:, 224:448], x_d[i0 + 1, r0:r0 + 113, :])
                    Xv = X[:, :].rearrange("p (i c) -> p i c", i=2)

                    D = wp.tile([113, 448], F32, tag="D")
                    Dv = D[:, :].rearrange("p (i c) -> p i c", i=2)
                    nc.gpsimd.memset(Dv[:, :, 0:1], 0.0)
                    nc.gpsimd.memset(Dv[:, :, 223:224], 0.0)
                    nc.vector.tensor_sub(Dv[:, :, 1:223], Xv[:, :, 0:222],
                                         Xv[:, :, 2:224])

                    S = wp.tile([113, 448], F32, tag="S")
                    Sv = S[:, :].rearrange("p (i c) -> p i c", i=2)
                    nc.vector.scalar_tensor_tensor(
                        Sv[:, :, 1:223], Xv[:, :, 1:223], 2.0, Xv[:, :, 0:222],
                        OP.mult, OP.add)
                    nc.vector.tensor_add(Sv[:, :, 1:223], Sv[:, :, 1:223],
                                         Xv[:, :, 2:224])
                    nc.gpsimd.tensor_add(Sv[:, :, 0:1], Xv[:, :, 0:1], Xv[:, :, 1:2])
                    nc.gpsimd.tensor_scalar_mul(Sv[:, :, 0:1], Sv[:, :, 0:1], 2.0)
                    nc.gpsimd.tensor_add(Sv[:, :, 223:224], Xv[:, :, 222:223],
                                         Xv[:, :, 223:224])
                    nc.gpsimd.tensor_scalar_mul(Sv[:, :, 223:224],
                                                Sv[:, :, 223:224], 2.0)

                    gxp = pp.tile([112, 448], F32, tag="gx")
                    gyp = pp.tile([112, 448], F32, tag="gy")
                    nc.tensor.matmul(gxp[:, :], lhs_s[ch], D[:, :],
                                     start=True, stop=True)
                    nc.tensor.matmul(gyp[:, :], lhs_d[ch], S[:, :],
                                     start=True, stop=True)

                    gys = wp.tile([112, 448], F32, tag="gys")
                    nc.scalar.activation(gys[:, :], gyp[:, :], AF.Copy)
                    A = wp.tile([112, 448], F32, tag="A")
                    nc.scalar.activation(A[:, :], gxp[:, :], AF.Square,
                                         bias=zb[0:112, 0:1])
                    C = wp.tile([112, 448], F32, tag="C")
                    nc.scalar.activation(C[:, :], gyp[:, :], AF.Square,
                                         bias=zb[0:112, 0:1])
                    Bt = wp.tile([112, 448], F32, tag="Bt")
                    nc.vector.tensor_mul(Bt[:, :], gxp[:, :], gys[:, :])
                    S2 = wp.tile([112, 448], F32, tag="S2")
                    nc.gpsimd.tensor_add(S2[:, :], A[:, :], C[:, :])
                    mg = wp.tile([112, 448], F32, tag="mg")
                    nc.scalar.activation(mg[:, :], S2[:, :], AF.Sqrt,
                                         bias=zb[0:112, 0:1], scale=gr2_ap)
                    flush_norm()
                    magG = wp.tile([112, 448], F32, tag="magG")
                    nc.vector.tensor_mul(magG[:, :], mg[:, :],
                                         _rep(gc_ap, 2))
                    magG16 = wp.tile([112, 448], BF16, tag="magG16")
                    nc.vector.tensor_copy(magG16[:, :], magG[:, :])

                    SG = wp.tile([112, 8 * 448], BF16, tag="SG")
                    for k in range(1, NB):
                        tk = TANS[k - 1]
                        q = wp.tile([112, 448], F32, tag="q")
                        eng = nc.vector
                        if k <= 4:
                            eng.scalar_tensor_tensor(
                                q[:, :], Bt[:, :], -tk, A[:, :], OP.mult, OP.add)
                        else:
                            eng.scalar_tensor_tensor(
                                q[:, :], Bt[:, :], tk, A[:, :], OP.mult, OP.subtract)
                        nc.scalar.activation(SG[:, (k - 1) * 448:k * 448],
                                             q[:, :], AF.Sigmoid,
                                             bias=nb40[0:112, 0:1], scale=1e30)
                    nc.vector.tensor_mul(SG[:, :], SG[:, :],
                                         _rep(magG16[:, :], 8))

                    CP = wp.tile([112, NB * 56], F32, tag="CP")
                    cpv = CP[:, :].rearrange("p (i k c) -> p k i c", i=2, k=NB)
                    nc.vector.reduce_sum(
                        cpv[:, 0:1, :, :].rearrange("p k i c -> p (k i) c"),
                        magG[:, :].rearrange("p (i c e) -> p i c e", i=2, e=8),
                        axis=AX)
                    nc.vector.reduce_sum(
                        cpv[:, 1:NB, :, :],
                        SG[:, :].rearrange("p (k i c e) -> p k i c e",
                                           k=8, i=2, e=8),
                        axis=AX)

                    Pp = pp2.tile([14, NB * 56], F32, tag="Pp")
                    nc.tensor.matmul(Pp[:, :], poolm_ap, CP[:, :],
                                     start=True, stop=True)
                    Ps = sp.tile([14, NB * 56], F32, tag="Ps")
                    nc.vector.tensor_copy(Ps[:, :], Pp[:, :])
                    psv = Ps[:, :].rearrange("p (i k c) -> p i k c", i=2, k=NB)
                    Hh = sp.tile([14, NB * 56], F32, tag="Hh")
                    hhv = Hh[:, :].rearrange("p (i k c) -> p i k c", i=2, k=NB)
                    nc.vector.tensor_sub(hhv[:, :, 0:8, :], psv[:, :, 0:8, :],
                                         psv[:, :, 1:9, :])
                    nc.vector.tensor_copy(hhv[:, :, 8, :], psv[:, :, 8, :])
                    sq = sp.tile([14, NB * 56], F32, tag="sq")
                    nc.gpsimd.tensor_mul(sq[:, :], Hh[:, :], Hh[:, :])
                    ss = sp.tile([14, 56], F32, tag="ss")
                    nc.vector.reduce_sum(
                        ss[:, :].rearrange("p (i c) -> p i c", i=2),
                        sq[:, :].rearrange("p (i k c) -> p i c k", i=2, k=NB),
                        axis=AX)
                    pending[0] = (Hh, ss, i0, ch)
            flush_norm()
    nc.compile()
    return nc


def _install_ntff_shim():
    """Provide antenv.axon_hooks (absent in this image) so trace=True works."""
    import sys as _sys
    if "antenv.axon_hooks" in _sys.modules:
        return
    import contextlib
    import ctypes
    import types

    so_path = "/opt/axon/libaxon_pjrt.so"
    lib = ctypes.CDLL(so_path)
    if not hasattr(lib, "axon_start_nrt_profile"):
        hook = None
    else:
        lib.axon_start_nrt_profile.argtypes = [
            ctypes.POINTER(ctypes.c_int64), ctypes.c_size_t]
        lib.axon_start_nrt_profile.restype = ctypes.c_int64
        lib.axon_stop_nrt_profile.argtypes = [ctypes.c_char_p]
        lib.axon_stop_nrt_profile.restype = ctypes.c_int64

        @contextlib.contextmanager
        def hook(output_dir, device_ids):
            import jax
            jax.devices()
            if device_ids:
                ids = (ctypes.c_int64 * len(device_ids))(*device_ids)
                rc = lib.axon_start_nrt_profile(ids, len(device_ids))
            else:
                rc = lib.axon_start_nrt_profile(None, 0)
            if rc != 0:
                raise RuntimeError(f"axon_start_nrt_profile rc={rc}")
            try:
                yield
            finally:
                n = lib.axon_stop_nrt_profile(str(output_dir).encode())
                print(f"profile: {n} file(s) written to {output_dir}",
                      file=sys.stderr)

    mod = types.ModuleType("antenv.axon_hooks")
    mod._hook = hook
    mod.get_axon_ntff_profile_hook = lambda: mod._hook
    mod.set_axon_ntff_profile_hook = lambda h: setattr(mod, "_hook", h)
    _sys.modules["antenv.axon_hooks"] = mod


_prog_cache = {}


def _get_prog(n_img):
    if n_img not in _prog_cache:
        _prog_cache[n_img] = build_program(n_img)
    return _prog_cache[n_img]


def kernel(x, weight_x, weight_y, gaussian_kernel, _trace=False):
    x = np.ascontiguousarray(np.asarray(x, np.float32).reshape(128, 224, 224))
    consts = _host_constants(weight_x, gaussian_kernel)
    nc = _get_prog(IMGS_PER_CORE)
    in_maps = []
    for c in range(N_CORES):
        m = {"x": x[c * IMGS_PER_CORE:(c + 1) * IMGS_PER_CORE]}
        m.update(consts)
        in_maps.append(m)
    if _trace:
        _install_ntff_shim()
    res = run_bass_kernel_spmd(nc, in_maps, core_ids=list(range(N_CORES)),
                               trace=_trace)
    outs = [r["out"] for r in res.results]            # (16, 28, 9, 28) each
    full = np.concatenate(outs, axis=0)               # (128, 28, 9, 28)
    feat = full.transpose(0, 2, 1, 3)                 # (b, 9, 28, 28)
    feat = feat.transpose(0, 2, 3, 1)                 # (b, 28, 28, 9)
    feat = feat.reshape(128, 14, 2, 14, 2, NB)
    feat = feat.transpose(0, 1, 3, 5, 2, 4).reshape(128, 196, NB * 4)
    if _trace:
        return np.ascontiguousarray(feat), res
    return np.ascontiguousarray(feat)



# revision 20
# speedup vs baseline: 1.2632x; 1.2632x over previous
"""HOG generator kernel for Trainium2, data-parallel over 8 NeuronCores.

v3: ratio + ACT-Sign binning.  Per tile = ONE image as [113p, (2, 224)f]
(top|bottom halves side by side).  Sobel horizontal on DVE fp16
tensor_tensor (2x mode), vertical on PE fp16 banded matmuls.  The 9
orientation boundaries are sign tests of rho = gx/gy against tan
constants: each mask is ONE single-input ACT op s_j = Sign(rho - tan_j)
in {-1,0,1} (constant as per-partition bias), freeing the DVE.  Signed
masked magnitudes V_j = sum(magG/2 * s_j) pool 8:1 (DVE reduce + Pool
pairwise-add tree), rows via PE matmul.  Adjacent differences V_j -
V_{j+1} give the bin histograms directly (the +-1 offset cancels), the
wrap bin is 4*P' + V_4 - V_{-4}, and the uniform x2 scale cancels in the
L2 normalization.  fp16 device output; host converts + unfolds.
"""
import math
import sys

import numpy as np

sys.path.insert(0, "/opt/trn_rl_repo")

import concourse.bass as bass
import concourse.bacc as bacc
import concourse.mybir as mybir
from concourse import tile
from concourse.bass_utils import run_bass_kernel_spmd

N_CORES = 8
IMGS_PER_CORE = 16
NB = 9
F32 = mybir.dt.float32
F16 = mybir.dt.float16
AF = mybir.ActivationFunctionType
OP = mybir.AluOpType
TANS9 = [math.tan(j * math.pi / 9.0) for j in range(-4, 5)]
DVE_PLANES = 4   # planes reduced via DVE tensor_reduce; rest on Pool tree


def _host_constants(weight_x, gaussian_kernel):
    g2 = np.asarray(gaussian_kernel, np.float64).reshape(16, 16)
    wt = np.sqrt(np.diag(g2)).astype(np.float32)   # g2[i,j] == wt[i]*wt[j]
    wx = np.asarray(weight_x, np.float32).reshape(3, 3)
    v_s = wx[:, 0].copy()                      # [1,2,1] vertical smooth
    v_d = wx[0, :].copy()                      # [1,0,-1] vertical diff

    def band(chunk, vec):
        m = np.zeros((113, 112), np.float32)
        for i in range(112):
            for dd in range(3):
                if chunk == 0:
                    r = i - 1 + dd
                    if r == -1:
                        r = 1
                else:
                    r = i + dd
                    if r == 113:
                        r = 111
                m[r, i] += vec[dd]
        return m

    poolm = np.zeros((112, 14), np.float32)
    for r in range(112):
        poolm[r, r // 8] = 1.0

    # half gaussian plane grc'[r, (h, c)] = wt_r * wt_c / 2  (row period 16;
    # both image halves share the row phase since 112 % 16 == 0)
    wr = wt[np.arange(112) % 16]
    wc = wt[np.arange(224) % 16]
    grch = 0.5 * wr[:, None] * np.tile(wc, 2)[None, :]   # [112, 448]

    c16 = np.zeros((113, 910), np.float16)
    c16[:, 0:112] = band(0, v_s)
    c16[:, 112:224] = band(1, v_s)
    c16[:, 224:336] = band(0, v_d)
    c16[:, 336:448] = band(1, v_d)
    c16[0:112, 448:462] = poolm
    c16[0:112, 462:910] = grch

    c32 = np.zeros((113, 10), np.float32)
    c32[0:14, 0] = 1e-24                      # eps bias for the norm sqrt
    for j in range(9):
        c32[:, 1 + j] = 1e-4 - TANS9[j]       # Sign bias (+delta tilts the
        # rho=0 boundary so exact-zero gx edge columns bin to 0, not half-8)
    return {"c16": c16, "c32": c32}


def _ap(t_ap, dims, off=0):
    """Build an AP on the same tensor with explicit [step, num] dims."""
    return bass.AP(t_ap.tensor, t_ap.offset + off, [list(d) for d in dims])


def build_program(n_img=IMGS_PER_CORE):
    nc = bacc.Bacc("TRN2", debug=False)
    x_d = nc.dram_tensor("x", [n_img, 224, 224], F32, kind="ExternalInput").ap()
    c16_d = nc.dram_tensor("c16", [113, 910], F16, kind="ExternalInput").ap()
    c32_d = nc.dram_tensor("c32", [113, 10], F32, kind="ExternalInput").ap()
    out_d = nc.dram_tensor("out", [n_img, 28, NB, 28], F16,
                           kind="ExternalOutput").ap()
    AX = mybir.AxisListType.X
    nd = DVE_PLANES
    npl = 10 - nd                      # planes on the Pool pairwise tree

    with tile.TileContext(nc) as tc:
        with (
            tc.tile_pool(name="const", bufs=1) as cp,
            tc.tile_pool(name="xin", bufs=3) as xp,
            tc.tile_pool(name="work", bufs=2) as wp,
            tc.tile_pool(name="big", bufs=2) as bp,
            tc.tile_pool(name="small", bufs=3) as sp,
            tc.tile_pool(name="psum", bufs=2, space="PSUM") as pp,
            tc.tile_pool(name="psum2", bufs=2, space="PSUM") as pp2,
        ):
            CT = cp.tile([113, 910], F16, tag="CT")
            nc.sync.dma_start(CT[:, :], c16_d)
            C32 = cp.tile([113, 10], F32, tag="C32")
            nc.sync.dma_start(C32[:, :], c32_d)
            bs = [CT[:, 0:112], CT[:, 112:224]]
            bd = [CT[:, 224:336], CT[:, 336:448]]
            poolm_ap = CT[0:112, 448:462]
            grch_ap = CT[0:112, 462:910]       # [112, 448] gaussian/2 plane
            eps_ap = C32[0:14, 0:1]

            for i0 in range(n_img):
                # ---- load image: halves side by side [113, (2, 224)] ----
                X = xp.tile([113, 448], F32, tag="X")
                nc.sync.dma_start(X[:, 0:224], x_d[i0, 0:113, :])
                nc.scalar.dma_start(X[:, 224:448], x_d[i0, 111:224, :])

                X16 = wp.tile([113, 448], F16, tag="X16")
                nc.vector.tensor_copy(X16[:, :], X[:, :])
                Xv = X16[:, :].rearrange("p (h c) -> p h c", h=2)

                # ---- horizontal sobel parts (fp16 TT, 2x) ----
                D = wp.tile([113, 448], F16, tag="D")
                Dv = D[:, :].rearrange("p (h c) -> p h c", h=2)
                nc.gpsimd.memset(Dv[:, :, 0:1], 0.0)
                nc.gpsimd.memset(Dv[:, :, 223:224], 0.0)
                nc.vector.tensor_sub(Dv[:, :, 1:223], Xv[:, :, 0:222],
                                     Xv[:, :, 2:224])

                S = wp.tile([113, 448], F16, tag="S")
                Sv = S[:, :].rearrange("p (h c) -> p h c", h=2)
                nc.vector.scalar_tensor_tensor(
                    Sv[:, :, 1:223], Xv[:, :, 1:223], 2.0, Xv[:, :, 0:222],
                    OP.mult, OP.add)
                nc.vector.tensor_add(Sv[:, :, 1:223], Sv[:, :, 1:223],
                                     Xv[:, :, 2:224])
                nc.gpsimd.tensor_add(Sv[:, :, 0:1], Xv[:, :, 0:1],
                                     Xv[:, :, 1:2])
                nc.gpsimd.tensor_scalar_mul(Sv[:, :, 0:1], Sv[:, :, 0:1], 2.0)
                nc.gpsimd.tensor_add(Sv[:, :, 223:224], Xv[:, :, 222:223],
                                     Xv[:, :, 223:224])
                nc.gpsimd.tensor_scalar_mul(Sv[:, :, 223:224],
                                            Sv[:, :, 223:224], 2.0)

                # ---- vertical sobel parts on PE (fp16 banded matmuls) ----
                gxp = pp.tile([112, 448], F32, tag="gx")
                gyp = pp.tile([112, 448], F32, tag="gy")
                for h in range(2):
                    nc.tensor.matmul(gxp[:, h * 224:(h + 1) * 224], bs[h],
                                     D[:, h * 224:(h + 1) * 224],
                                     start=True, stop=True)
                    nc.tensor.matmul(gyp[:, h * 224:(h + 1) * 224], bd[h],
                                     S[:, h * 224:(h + 1) * 224],
                                     start=True, stop=True)

                # ---- ratio rho = gx/gy, magnitude ----
                gx16 = wp.tile([112, 448], F16, tag="gx16")
                nc.scalar.activation(gx16[:, :], gxp[:, :], AF.Copy, bias=1e-6)
                A = wp.tile([112, 448], F16, tag="A")
                nc.scalar.activation(A[:, :], gxp[:, :], AF.Square)
                Cq = wp.tile([112, 448], F16, tag="Cq")
                nc.scalar.activation(Cq[:, :], gyp[:, :], AF.Square)
                rgy = wp.tile([112, 448], F16, tag="rgy")
                with nc.allow_low_precision("ratio in fp16"):
                    nc.vector.reciprocal(rgy[:, :], gyp[:, :])
                RHO = wp.tile([112, 448], F16, tag="RHO")
                nc.vector.tensor_mul(RHO[:, :], gx16[:, :], rgy[:, :])
                S2 = wp.tile([112, 448], F16, tag="S2")
                nc.vector.tensor_add(S2[:, :], A[:, :], Cq[:, :])
                mag = wp.tile([112, 448], F16, tag="mag")
                nc.scalar.activation(mag[:, :], S2[:, :], AF.Sqrt)

                # ---- 9 sign masks on ACT; SGALL planes 0..8 = magG'*s_j,
                # ---- plane 9 = magG' itself ----
                M = bp.tile([112, NB * 448], F16, tag="M")
                for j in range(NB):
                    nc.scalar.activation(M[:, j * 448:(j + 1) * 448],
                                         RHO[:, :], AF.Sign,
                                         bias=C32[0:112, 1 + j:2 + j])
                SGALL = bp.tile([112, 10 * 448], F16, tag="SGALL")
                magGh = SGALL[:, NB * 448:]
                nc.vector.tensor_mul(magGh, mag[:, :], grch_ap)
                nc.vector.tensor_mul(
                    SGALL[:, 0:NB * 448],
                    M[:, :],
                    _ap(magGh, [magGh.ap[0], [0, NB], [1, 448]]))

                # ---- 8:1 column pooling: nd planes DVE reduce, npl on a
                # ---- 3-level pairwise-add tree on Pool ----
                CP = sp.tile([112, 10 * 56], F16, tag="CP")
                with nc.allow_low_precision("pooled sums in fp16"):
                    nc.vector.reduce_sum(
                        CP[:, 0:nd * 56],
                        SGALL[:, 0:nd * 448].rearrange(
                            "p (k m e) -> p (k m) e", k=nd, e=8),
                        axis=AX)
                sg = SGALL[:, nd * 448:]
                T1 = sp.tile([112, npl * 224], F16, tag="T1")
                T2 = sp.tile([112, npl * 112], F16, tag="T2")
                nc.gpsimd.tensor_add(
                    _ap(T1[:, :], [T1[:, :].ap[0], [224, npl], [4, 56],
                                   [1, 4]]),
                    _ap(sg, [sg.ap[0], [448, npl], [8, 56], [2, 4]]),
                    _ap(sg, [sg.ap[0], [448, npl], [8, 56], [2, 4]], off=1))
                nc.gpsimd.tensor_add(
                    _ap(T2[:, :], [T2[:, :].ap[0], [112, npl], [2, 56],
                                   [1, 2]]),
                    _ap(T1[:, :], [T1[:, :].ap[0], [224, npl], [4, 56],
                                   [2, 2]]),
                    _ap(T1[:, :], [T1[:, :].ap[0], [224, npl], [4, 56],
                                   [2, 2]], off=1))
                nc.gpsimd.tensor_add(
                    _ap(CP[:, :], [CP[:, :].ap[0], [56, npl], [1, 56]],
                        off=nd * 56),
                    _ap(T2[:, :], [T2[:, :].ap[0], [112, npl], [2, 56]]),
                    _ap(T2[:, :], [T2[:, :].ap[0], [112, npl], [2, 56]],
                        off=1))

                # ---- 8:1 row pooling on PE (two PSUM banks) ----
                Pp1 = pp2.tile([14, NB * 56], F32, tag="Pp1")
                nc.tensor.matmul(Pp1[:, :], poolm_ap, CP[:, 0:NB * 56],
                                 start=True, stop=True)
                Pp2 = pp2.tile([14, 56], F32, tag="Pp2")
                nc.tensor.matmul(Pp2[:, :], poolm_ap, CP[:, NB * 56:],
                                 start=True, stop=True)

                # ---- histograms: bin(5..8,0..3) = V_j - V_{j+1};
                # ---- bin4 = 4P' + V_4 - V_{-4}; all bins scaled x2 (the
                # ---- L2 normalization cancels the scale) ----
                Psf = sp.tile([14, 8 * 56], F32, tag="Psf")
                nc.scalar.activation(Psf[:, :], Pp1[:, 56:NB * 56], AF.Copy)
                Psf2 = sp.tile([14, 56], F32, tag="Psf2")
                nc.scalar.activation(Psf2[:, :], Pp2[:, :], AF.Copy)
                Hh = sp.tile([14, NB * 56], F16, tag="Hh")
                nc.vector.tensor_sub(Hh[:, 5 * 56:NB * 56], Pp1[:, 0:224],
                                     Psf[:, 0:224])
                nc.vector.tensor_sub(Hh[:, 0:4 * 56], Pp1[:, 224:448],
                                     Psf[:, 224:448])
                t1 = sp.tile([14, 56], F16, tag="t1")
                nc.vector.tensor_sub(t1[:, :], Psf[:, 7 * 56:8 * 56],
                                     Pp1[:, 0:56])
                nc.vector.scalar_tensor_tensor(
                    Hh[:, 4 * 56:5 * 56], Psf2[:, :], 2.0, t1[:, :],
                    OP.mult, OP.add)

                # ---- L2 normalize over the 9 bins ----
                sq = sp.tile([14, NB * 56], F16, tag="sq")
                nc.vector.tensor_mul(sq[:, :], Hh[:, :], Hh[:, :])
                ss = sp.tile([14, 56], F32, tag="ss")
                nc.vector.reduce_sum(
                    ss[:, :],
                    _ap(sq[:, :], [sq[:, :].ap[0], [1, 56], [56, NB]]),
                    axis=AX)
                nrm = sp.tile([14, 56], F32, tag="nrm")
                nc.scalar.activation(nrm[:, :], ss[:, :], AF.Sqrt,
                                     bias=eps_ap)
                inv = sp.tile([14, 56], F16, tag="inv")
                with nc.allow_low_precision("normalizer in fp16"):
                    nc.vector.reciprocal(inv[:, :], nrm[:, :])
                OUT = sp.tile([14, NB * 56], F16, tag="OUT")
                nc.vector.tensor_mul(
                    OUT[:, :].rearrange("p (k m) -> p k m", k=NB),
                    Hh[:, :].rearrange("p (k m) -> p k m", k=NB),
                    _ap(inv[:, :], [inv[:, :].ap[0], [0, NB], [1, 56]]))

                # ---- store: OUT [14, (k, h, c)] -> out[i, h*14+r, k, c] ----
                odst = bass.AP(out_d.tensor, out_d.offset + i0 * 28 * NB * 28,
                               [[NB * 28, 14], [28, NB], [14 * NB * 28, 2],
                                [1, 28]])
                nc.sync.dma_start(
                    odst, OUT[:, :].rearrange("p (k h c) -> p k h c",
                                              k=NB, h=2))
    nc.compile()
    return nc


def _install_ntff_shim():
    """Provide antenv.axon_hooks (absent in this image) so trace=True works."""
    import sys as _sys
    if "antenv.axon_hooks" in _sys.modules:
        return
    import contextlib
    import ctypes
    import types

    so_path = "/opt/axon/libaxon_pjrt.so"
    lib = ctypes.CDLL(so_path)
    if not hasattr(lib, "axon_start_nrt_profile"):
        hook = None
    else:
        lib.axon_start_nrt_profile.argtypes = [
            ctypes.POINTER(ctypes.c_int64), ctypes.c_size_t]
        lib.axon_start_nrt_profile.restype = ctypes.c_int64
        lib.axon_stop_nrt_profile.argtypes = [ctypes.c_char_p]
        lib.axon_stop_nrt_profile.restype = ctypes.c_int64

        @contextlib.contextmanager
        def hook(output_dir, device_ids):
            import jax
            jax.devices()
            if device_ids:
                ids = (ctypes.c_int64 * len(device_ids))(*device_ids)
                rc = lib.axon_start_nrt_profile(ids, len(device_ids))
            else:
                rc = lib.axon_start_nrt_profile(None, 0)
            if rc != 0:
                raise RuntimeError(f"axon_start_nrt_profile rc={rc}")
            try:
                yield
            finally:
                n = lib.axon_stop_nrt_profile(str(output_dir).encode())
                print(f"profile: {n} file(s) written to {output_dir}",
                      file=sys.stderr)

    mod = types.ModuleType("antenv.axon_hooks")
    mod._hook = hook
    mod.get_axon_ntff_profile_hook = lambda: mod._hook
    mod.set_axon_ntff_profile_hook = lambda h: setattr(mod, "_hook", h)
    _sys.modules["antenv.axon_hooks"] = mod


_prog_cache = {}


def _get_prog(n_img):
    if n_img not in _prog_cache:
        _prog_cache[n_img] = build_program(n_img)
    return _prog_cache[n_img]


def kernel(x, weight_x, weight_y, gaussian_kernel, _trace=False):
    x = np.ascontiguousarray(np.asarray(x, np.float32).reshape(128, 224, 224))
    consts = _host_constants(weight_x, gaussian_kernel)
    nc = _get_prog(IMGS_PER_CORE)
    in_maps = []
    for c in range(N_CORES):
        m = {"x": x[c * IMGS_PER_CORE:(c + 1) * IMGS_PER_CORE]}
        m.update(consts)
        in_maps.append(m)
    if _trace:
        _install_ntff_shim()
    res = run_bass_kernel_spmd(nc, in_maps, core_ids=list(range(N_CORES)),
                               trace=_trace)
    outs = [np.asarray(r["out"], np.float32) for r in res.results]
    full = np.concatenate(outs, axis=0)                # (128, 28, 9, 28)
    feat = full.transpose(0, 2, 1, 3)                  # (b, 9, 28, 28)
    feat = feat.transpose(0, 2, 3, 1)                  # (b, 28, 28, 9)
    feat = feat.reshape(128, 14, 2, 14, 2, NB)
    feat = feat.transpose(0, 1, 3, 5, 2, 4).reshape(128, 196, NB * 4)
    if _trace:
        return np.ascontiguousarray(feat), res
    return np.ascontiguousarray(feat)
